# revision 7
# baseline (speedup 1.0000x reference)
"""Depthwise cross-correlation (DepthwiseRPN) on 8 TRN2 NeuronCores.

Reference op:
  z_f: [B=128, C=256, 7, 7]   per-(b,c) kernels
  x_f: [B=128, C=256, 31, 31] search windows
  out: [B=128, C=256, 25, 25] valid cross-correlation per (b,c)

Sharding: pure data-parallel over B (16 batches per core).

Depthwise conv has no operand shared across a matmul grid, so TensorE
can only do ~128 useful MACs/cycle (diagonal weights; rhs-ingest
bound).  To beat the PE-only floor (~420 us/core) the per-core work is
split across three parallel pipelines by channel group (128 ch each):

  - PE groups:  per-tap diagonal matmul, 49 taps accumulate in PSUM.
    psum[c,:] += diag(z[:,u,v]) @ x[:, shifted-window AP].
  - DVE groups: fused MAC via the AFFINE_THEN_ADD custom DVE op:
    acc = x_win*z_tap + acc  (bf16 reads, fp32 accumulator).
  - ACT+GpSimd groups: ScalarE mult (activation Copy with per-partition
    scale) into tmp, GpSimd tensor_add accumulates.

Diag lhsT matrices are precomputed host-side (bf16); shifted windows
are pure access patterns on SBUF tiles (no data movement).
"""

import numpy as np
import ml_dtypes

import concourse.bass as bass
import concourse.mybir as mybir
import concourse.tile as tile
from concourse import bacc
from concourse.bass_utils import run_bass_kernel_spmd

B, C = 128, 256
HX, WX = 31, 31
HZ, WZ = 7, 7
HO, WO = HX - HZ + 1, WX - WZ + 1  # 25, 25
NCORES = 8
BPC = B // NCORES         # batches per core = 16
Q = BPC * C               # (b,c) channels per core = 4096
G = Q // 128              # groups of 128 channels = 32
NX = HX * WX              # 961
NO = HO * WO              # 625
NT = HZ * WZ              # 49 taps
ROWS_A = 20               # psum chunk A rows (20*25=500 <= 512)
ROWS_B = HO - ROWS_A      # 5 rows (125)

# channel-group split across engines
G_PE = 21
G_DVE = 7
G_AG = G - G_PE - G_DVE   # 4

BF16 = ml_dtypes.bfloat16

_built = {}


def _ensure_ntff_hook():
    """Install the axon NTFF profiling hook if the container's antenv stub
    lacks it (needed only for trace=True local profiling runs)."""
    import contextlib
    import ctypes
    import sys
    import types

    try:
        from antenv.axon_hooks import get_axon_ntff_profile_hook  # noqa: F401

        return True
    except ImportError:
        pass
    so_path = "/opt/axon/libaxon_pjrt.so"
    try:
        lib = ctypes.CDLL(so_path)
    except OSError:
        return False
    if not hasattr(lib, "axon_start_nrt_profile"):
        return False
    lib.axon_start_nrt_profile.argtypes = [
        ctypes.POINTER(ctypes.c_int64),
        ctypes.c_size_t,
    ]
    lib.axon_start_nrt_profile.restype = ctypes.c_int64
    lib.axon_stop_nrt_profile.argtypes = [ctypes.c_char_p]
    lib.axon_stop_nrt_profile.restype = ctypes.c_int64

    @contextlib.contextmanager
    def _hook(output_dir, device_ids):
        import jax

        jax.devices()
        if device_ids:
            ids = (ctypes.c_int64 * len(device_ids))(*device_ids)
            rc = lib.axon_start_nrt_profile(ids, len(device_ids))
        else:
            rc = lib.axon_start_nrt_profile(None, 0)
        if rc != 0:
            raise RuntimeError(f"axon_start_nrt_profile rc={rc}")
        try:
            yield
        finally:
            n = lib.axon_stop_nrt_profile(str(output_dir).encode())
            print(f"profile: {n} file(s) written to {output_dir}", file=sys.stderr)

    state = {"hook": _hook}
    mod = types.ModuleType("antenv.axon_hooks")
    mod.get_axon_ntff_profile_hook = lambda: state["hook"]
    mod.set_axon_ntff_profile_hook = lambda h: state.update(hook=h)
    import antenv

    sys.modules["antenv.axon_hooks"] = mod
    antenv.axon_hooks = mod
    return True


def _emit_pe_group(nc, pools, x_d, zd_d, out_d, g):
    xp, zp, op, psA, psB = pools["xp"], pools["zp"], pools["op"], pools["psA"], pools["psB"]
    x_sb = xp.tile([128, HX, WX], mybir.dt.bfloat16, name=f"xpe{g}", tag="xpe")
    zd_sb = zp.tile([128, NT, 128], mybir.dt.bfloat16, name=f"zd{g}", tag="zd")
    nc.sync.dma_start(out=x_sb, in_=x_d[g].rearrange("p (h w) -> p h w", h=HX))
    nc.sync.dma_start(out=zd_sb, in_=zd_d[g])

    pA = psA.tile([128, ROWS_A * WO], mybir.dt.float32, name=f"pA{g}", tag="pA")
    pB = psB.tile([128, ROWS_B * WO], mybir.dt.float32, name=f"pB{g}", tag="pB")
    for t in range(NT):
        u, v = divmod(t, WZ)
        lhsT = zd_sb[:, t, :]
        nc.tensor.matmul(
            pA[:, :], lhsT, x_sb[:, u : u + ROWS_A, v : v + WO],
            start=(t == 0), stop=(t == NT - 1),
        )
        nc.tensor.matmul(
            pB[:, :], lhsT, x_sb[:, ROWS_A + u : ROWS_A + u + ROWS_B, v : v + WO],
            start=(t == 0), stop=(t == NT - 1),
        )

    out_sb = op.tile([128, NO], mybir.dt.float32, name=f"ope{g}", tag="ope")
    # ScalarE is closest to PSUM; keep DVE free for its MAC pipeline
    nc.scalar.copy(out=out_sb[:, : ROWS_A * WO], in_=pA[:, :])
    nc.scalar.copy(out=out_sb[:, ROWS_A * WO :], in_=pB[:, :])
    nc.sync.dma_start(out=out_d[g], in_=out_sb)


def _emit_dve_group(nc, pools, x_d, zf_d, out_d, g):
    xp, zp, op = pools["xv"], pools["zf"], pools["ov"]
    x_sb = xp.tile([128, HX, WX], mybir.dt.bfloat16, name=f"xdv{g}", tag="xdv")
    zf_sb = zp.tile([128, NT], mybir.dt.float32, name=f"zfv{g}", tag="zfv")
    nc.sync.dma_start(out=x_sb, in_=x_d[g].rearrange("p (h w) -> p h w", h=HX))
    nc.sync.dma_start(out=zf_sb, in_=zf_d[g - G_PE])

    acc = op.tile([128, HO, WO], mybir.dt.float32, name=f"accv{g}", tag="accv")
    for t in range(NT):
        u, v = divmod(t, WZ)
        win = x_sb[:, u : u + HO, v : v + WO]
        if t == 0:
            nc.vector.tensor_scalar(
                acc, win, zf_sb[:, 0:1], None, mybir.AluOpType.mult
            )
        else:
            nc.vector.affine_then_add(acc, win, acc, zf_sb[:, t : t + 1], 0.0)
    nc.sync.dma_start(out=out_d[g], in_=acc.rearrange("p h w -> p (h w)"))


def _emit_ag_group(nc, pools, x_d, zf_d, out_d, g):
    xp, zp, op, tp = pools["xa"], pools["zfa"], pools["oa"], pools["ta"]
    x_sb = xp.tile([128, HX, WX], mybir.dt.bfloat16, name=f"xag{g}", tag="xag")
    zf_sb = zp.tile([128, NT], mybir.dt.float32, name=f"zfa{g}", tag="zfa")
    nc.sync.dma_start(out=x_sb, in_=x_d[g].rearrange("p (h w) -> p h w", h=HX))
    nc.sync.dma_start(out=zf_sb, in_=zf_d[g - G_PE])

    acc = op.tile([128, HO, WO], mybir.dt.bfloat16, name=f"acca{g}", tag="acca")
    outf = op.tile([128, NO], mybir.dt.float32, name=f"outa{g}", tag="outa")
    for t in range(NT):
        u, v = divmod(t, WZ)
        win = x_sb[:, u : u + HO, v : v + WO]
        if t == 0:
            nc.scalar.activation(
                acc, win, mybir.ActivationFunctionType.Copy,
                bias=0.0, scale=zf_sb[:, 0:1],
            )
        else:
            tmp = tp.tile([128, HO, WO], mybir.dt.bfloat16, name=f"tmpa{g}_{t}", tag="tmpa")
            nc.scalar.activation(
                tmp, win, mybir.ActivationFunctionType.Copy,
                bias=0.0, scale=zf_sb[:, t : t + 1],
            )
            nc.gpsimd.tensor_add(acc, acc, tmp)
    nc.scalar.copy(out=outf, in_=acc.rearrange("p h w -> p (h w)"))
    nc.sync.dma_start(out=out_d[g], in_=outf)


def _build():
    """Build + compile the SPMD Bass program (cached per process)."""
    if "nc" in _built:
        return _built["nc"]

    nc = bacc.Bacc(
        "TRN2", target_bir_lowering=False, debug=False, num_devices=NCORES
    )
    x_d = nc.dram_tensor("x", [G, 128, NX], mybir.dt.bfloat16, kind="ExternalInput").ap()
    zd_d = nc.dram_tensor(
        "zd", [G_PE, 128, NT, 128], mybir.dt.bfloat16, kind="ExternalInput"
    ).ap()
    zf_d = nc.dram_tensor(
        "zf", [G - G_PE, 128, NT], mybir.dt.float32, kind="ExternalInput"
    ).ap()
    out_d = nc.dram_tensor("out", [G, 128, NO], mybir.dt.float32, kind="ExternalOutput").ap()

    with tile.TileContext(nc) as tc:
        with (
            tc.tile_pool(name="xp", bufs=2) as xp,
            tc.tile_pool(name="zp", bufs=2) as zp,
            tc.tile_pool(name="op", bufs=2) as op,
            tc.tile_pool(name="xv", bufs=2) as xv,
            tc.tile_pool(name="zf", bufs=2) as zf,
            tc.tile_pool(name="ov", bufs=2) as ov,
            tc.tile_pool(name="xa", bufs=2) as xa,
            tc.tile_pool(name="zfa", bufs=2) as zfa,
            tc.tile_pool(name="oa", bufs=2) as oa,
            tc.tile_pool(name="ta", bufs=2) as ta,
            tc.tile_pool(name="psA", bufs=2, space="PSUM") as psA,
            tc.tile_pool(name="psB", bufs=2, space="PSUM") as psB,
        ):
            pools = dict(xp=xp, zp=zp, op=op, xv=xv, zf=zf, ov=ov,
                         xa=xa, zfa=zfa, oa=oa, ta=ta, psA=psA, psB=psB)
            # interleave pipeline emission so every engine has early work
            dve_iter = iter(range(G_PE, G_PE + G_DVE))
            ag_iter = iter(range(G_PE + G_DVE, G))
            for r in range(G_PE):
                _emit_pe_group(nc, pools, x_d, zd_d, out_d, r)
                if r % 3 == 0:
                    g = next(dve_iter, None)
                    if g is not None:
                        _emit_dve_group(nc, pools, x_d, zf_d, out_d, g)
                if r % 5 == 0:
                    g = next(ag_iter, None)
                    if g is not None:
                        _emit_ag_group(nc, pools, x_d, zf_d, out_d, g)
            for g in dve_iter:
                _emit_dve_group(nc, pools, x_d, zf_d, out_d, g)
            for g in ag_iter:
                _emit_ag_group(nc, pools, x_d, zf_d, out_d, g)

    nc.compile()
    _built["nc"] = nc
    return nc


def _host_prep(z_f: np.ndarray, x_f: np.ndarray):
    """Shard + reformat inputs for the 8 cores."""
    x = np.ascontiguousarray(x_f, dtype=np.float32).reshape(B, C, NX)
    z = np.ascontiguousarray(z_f, dtype=np.float32).reshape(B, C, NT)
    in_maps = []
    p_idx = np.arange(128)
    for k in range(NCORES):
        xs = x[k * BPC : (k + 1) * BPC].reshape(G, 128, NX).astype(BF16)
        zs = z[k * BPC : (k + 1) * BPC].reshape(G, 128, NT)
        zd = np.zeros((G_PE, 128, NT, 128), dtype=BF16)
        # zd[g, p, t, p] = z[g*128+p, t]
        zd[:, p_idx, :, p_idx] = zs[:G_PE].astype(BF16).transpose(1, 0, 2)
        zfl = np.ascontiguousarray(zs[G_PE:])  # fp32 for DVE/AG scalar slots
        in_maps.append({"x": xs, "zd": zd, "zf": zfl})
    return in_maps


def _run(z_f, x_f, trace=False, **spmd_kwargs):
    nc = _build()
    in_maps = _host_prep(z_f, x_f)
    if trace:
        _ensure_ntff_hook()
        # local profiling only — skip the artifact share upload
        import concourse.bass_utils as _bu

        _bu.upload_artifacts = lambda tmpdir: tmpdir
    res = run_bass_kernel_spmd(
        nc, in_maps, core_ids=list(range(NCORES)), trace=trace, **spmd_kwargs
    )
    full = np.empty((B, C, HO, WO), np.float32)
    fv = full.reshape(NCORES, Q, NO)
    for k, r in enumerate(res.results):
        fv[k] = np.asarray(r["out"], dtype=np.float32).reshape(Q, NO)
    return full, res


def kernel(z_f: np.ndarray, x_f: np.ndarray) -> np.ndarray:
    full, _ = _run(z_f, x_f, trace=False)
    return full
